# revision 1
# baseline (speedup 1.0000x reference)
"""ContrastiveLoss Trainium2 kernel.

Strategy (data-parallel over 8 NeuronCores):
  - 8 cores = 4 batches x 2 halves. Core c handles batch b=c//2, half h=c%2:
    2500 match pairs + 25000 non-match pairs.
  - Gather primitive: SWDGE vector-indirect DMA (`indirect_dma_start`), which
    on TRN2 fetches ONE dynamic row index per SBUF partition and streams the
    row (16 f32 = 64 B) into that partition. One instruction = 128 rows.
    Samples are column-blocked: sample s = block j * 128 + partition p, so
    block j's indices live in idx[:, j] and its rows land in g[:, 16j:16j+16].
  - Per-sample math on DVE/ACT (fully hidden under the gather stream):
      match partial  = sum((mA-mB)^2)              (DVE sub + fused sq-reduce)
      nonmatch partial = sum(relu(0.5-||nA-nB||^2)) (DVE sub, ACT square,
        DVE grouped reduce over D=16, ACT relu with fused accumulate)
  - Padding: tail samples use index 0 on both sides; a host-built {0,1} mask
    zeroes pad match diffs exactly, and a host-built additive bias pushes pad
    nonmatch distances to 1e9 so the hinge is exactly 0.
  - Partition reduction via a ones-vector TensorE matmul into PSUM.
  - Each core outputs [1,2] raw partial sums; the host combines 8x2 scalars
    and applies the 1/5000 and 1/50000 normalizations.

kernel() takes the FULL (unsharded) inputs and returns the full output tuple
(contrastive_loss_sum, match_loss_sum, nonmatch_loss_sum) like the reference.
"""

import os

import numpy as np

# Problem constants (hardcoded per task spec).
B, N, D = 4, 307200, 16
M_MATCH, M_NONMATCH = 5000, 50000
MARGIN = 0.5
NON_MATCH_WEIGHT = 1.0
NCORES = 8

P = 128
MH = M_MATCH // 2  # 2500 match samples per core
NH = M_NONMATCH // 2  # 25000 nonmatch samples per core
MBLK = (MH + P - 1) // P  # 20 match blocks (last one partial: 2500=19*128+68)
NBLK = (NH + P - 1) // P  # 196 nonmatch blocks (25000=195*128+40)
M_REM = MH - (MBLK - 1) * P  # 68 real rows in last match block
N_REM = NH - (NBLK - 1) * P  # 40 real rows in last nonmatch block
CBLK = 28  # nonmatch blocks per compute chunk
NCH = NBLK // CBLK  # 7 chunks
assert NCH * CBLK == NBLK

LAST_EXEC_NS = None

_CACHE = {}


def _build_nc():
    import concourse.bacc as bacc
    import concourse.mybir as mybir
    import concourse.tile as tile
    from concourse import bass

    f32 = mybir.dt.float32
    i32 = mybir.dt.int32
    X = mybir.AxisListType.X
    ADD = mybir.AluOpType.add
    MULT = mybir.AluOpType.mult
    Relu = mybir.ActivationFunctionType.Relu

    nc = bacc.Bacc("TRN2", target_bir_lowering=False, debug=False)
    eA = nc.dram_tensor("eA", (N, D), f32, kind="ExternalInput")
    eB = nc.dram_tensor("eB", (N, D), f32, kind="ExternalInput")
    imA = nc.dram_tensor("imA", (P, MBLK), i32, kind="ExternalInput")
    imB = nc.dram_tensor("imB", (P, MBLK), i32, kind="ExternalInput")
    inA = nc.dram_tensor("inA", (P, NBLK), i32, kind="ExternalInput")
    inB = nc.dram_tensor("inB", (P, NBLK), i32, kind="ExternalInput")
    # pad handling: mmask is 1.0 for real match samples else 0.0;
    # npad adds 1e9 to pad nonmatch distances (hinge -> exactly 0)
    mmask = nc.dram_tensor("mmask", (P, MBLK), f32, kind="ExternalInput")
    npad = nc.dram_tensor("npad", (P, CBLK), f32, kind="ExternalInput")
    out = nc.dram_tensor("out", (1, 2), f32, kind="ExternalOutput")

    def gather(dst_ap, src, idx_ap):
        nc.gpsimd.indirect_dma_start(
            out=dst_ap,
            out_offset=None,
            in_=src.ap(),
            in_offset=bass.IndirectOffsetOnAxis(ap=idx_ap, axis=0),
        )

    with tile.TileContext(nc) as tc:
        with (
            tc.tile_pool(name="idx", bufs=1) as idxp,
            tc.tile_pool(name="gath", bufs=3) as gp,
            tc.tile_pool(name="cmp", bufs=3) as cp,
            tc.tile_pool(name="sums", bufs=1) as sp,
            tc.tile_pool(name="psum", bufs=1, space="PSUM") as pp,
        ):
            # index tiles (HWDGE loads; keep Pool free for the gather stream)
            inA_t = idxp.tile([P, NBLK], i32)
            nc.sync.dma_start(inA_t[:], inA.ap())
            inB_t = idxp.tile([P, NBLK], i32)
            nc.sync.dma_start(inB_t[:], inB.ap())
            imA_t = idxp.tile([P, MBLK], i32)
            nc.sync.dma_start(imA_t[:], imA.ap())
            imB_t = idxp.tile([P, MBLK], i32)
            nc.sync.dma_start(imB_t[:], imB.ap())
            mmask_t = idxp.tile([P, MBLK], f32)
            nc.sync.dma_start(mmask_t[:], mmask.ap())
            npad_t = idxp.tile([P, CBLK], f32)
            nc.sync.dma_start(npad_t[:], npad.ap())

            sums = sp.tile([P, 1 + NCH], f32)
            margin_t = sp.tile([P, 1], f32)
            nc.vector.memset(margin_t[:], MARGIN)

            # --- nonmatch: gather + compute in chunks of CBLK blocks ---
            for c in range(NCH):
                ga = gp.tile([P, CBLK * D], f32, tag="ga")
                gb = gp.tile([P, CBLK * D], f32, tag="gb")
                for j in range(CBLK):
                    gj = c * CBLK + j
                    gather(ga[:, j * D : (j + 1) * D], eA, inA_t[:, gj : gj + 1])
                    gather(gb[:, j * D : (j + 1) * D], eB, inB_t[:, gj : gj + 1])

                nd = cp.tile([P, CBLK * D], f32, tag="nd")
                nc.vector.tensor_sub(nd[:], ga[:], gb[:])
                nsq = cp.tile([P, CBLK * D], f32, tag="nsq")
                nc.scalar.square(nsq[:], nd[:])
                dist = cp.tile([P, CBLK], f32, tag="dist")
                nc.vector.tensor_reduce(
                    dist[:],
                    nsq[:].rearrange("p (s d) -> p s d", d=D),
                    axis=X,
                    op=ADD,
                )
                if c == NCH - 1:
                    # pad samples: add 1e9 to their distance so the hinge
                    # is exactly 0
                    nc.vector.tensor_add(dist[:], dist[:], npad_t[:])
                hng = cp.tile([P, CBLK], f32, tag="hng")
                nc.scalar.activation(
                    hng[:],
                    dist[:],
                    Relu,
                    bias=margin_t[:],
                    scale=-1.0,
                    accum_out=sums[:, 1 + c : 2 + c],
                )

            # --- match: 20 blocks in one shot ---
            ma = gp.tile([P, MBLK * D], f32, tag="ma")
            mb = gp.tile([P, MBLK * D], f32, tag="mb")
            for j in range(MBLK):
                gather(ma[:, j * D : (j + 1) * D], eA, imA_t[:, j : j + 1])
                gather(mb[:, j * D : (j + 1) * D], eB, imB_t[:, j : j + 1])
            md = cp.tile([P, MBLK * D], f32, tag="md")
            nc.vector.tensor_sub(md[:], ma[:], mb[:])
            # mask the pad samples exactly: mdm = md * mmask (broadcast over D)
            mdm = cp.tile([P, MBLK * D], f32, tag="mdm")
            nc.vector.tensor_tensor(
                out=mdm[:].rearrange("p (s d) -> p s d", d=D),
                in0=md[:].rearrange("p (s d) -> p s d", d=D),
                in1=mmask_t[:].unsqueeze(2).to_broadcast([P, MBLK, D]),
                op=MULT,
            )
            msq = cp.tile([P, MBLK * D], f32, tag="msq")
            nc.scalar.activation(
                msq[:],
                mdm[:],
                mybir.ActivationFunctionType.Square,
                accum_out=sums[:, 0:1],
            )

            # --- cross-partition reduction: ones[128,1].T @ sums[128,1+NCH] ---
            ones = sp.tile([P, 1], f32)
            nc.vector.memset(ones[:], 1.0)
            acc = pp.tile([1, 1 + NCH], f32, space="PSUM")
            nc.tensor.matmul(acc[:], lhsT=ones[:], rhs=sums[:], start=True, stop=True)
            res = sp.tile([1, 2], f32)
            nc.vector.tensor_copy(res[:, 0:1], acc[:, 0:1])
            nc.vector.tensor_reduce(res[:, 1:2], acc[:, 1 : 1 + NCH], axis=X, op=ADD)
            nc.sync.dma_start(out.ap(), res[:])

    nc.compile()
    return nc


def _get_nc():
    if "nc" not in _CACHE:
        _CACHE["nc"] = _build_nc()
    return _CACHE["nc"]


def _blocked(idx_1d, nblocks):
    """[n] -> [128, nblocks] with sample s at [s % 128, s // 128]; pad with 0."""
    out = np.zeros((P, nblocks), np.int32)
    n = idx_1d.shape[0]
    full = n // P
    out[:, :full] = idx_1d[: full * P].reshape(full, P).T
    rem = n - full * P
    if rem:
        out[:rem, full] = idx_1d[full * P :]
    return out


def _in_maps(outA, outB, matchA, matchB, nonMatchA, nonMatchB):
    outA = np.ascontiguousarray(np.asarray(outA, dtype=np.float32))
    outB = np.ascontiguousarray(np.asarray(outB, dtype=np.float32))
    matchA = np.asarray(matchA).astype(np.int32)
    matchB = np.asarray(matchB).astype(np.int32)
    nonMatchA = np.asarray(nonMatchA).astype(np.int32)
    nonMatchB = np.asarray(nonMatchB).astype(np.int32)

    mmask = np.zeros((P, MBLK), np.float32)
    mmask[:, : MBLK - 1] = 1.0
    mmask[:M_REM, MBLK - 1] = 1.0
    npad = np.zeros((P, CBLK), np.float32)
    npad[N_REM:, CBLK - 1] = 1e9

    maps = []
    for c in range(NCORES):
        b, h = c // 2, c % 2
        maps.append(
            {
                "eA": outA[b],
                "eB": outB[b],
                "imA": _blocked(matchA[b, h * MH : (h + 1) * MH], MBLK),
                "imB": _blocked(matchB[b, h * MH : (h + 1) * MH], MBLK),
                "inA": _blocked(nonMatchA[b, h * NH : (h + 1) * NH], NBLK),
                "inB": _blocked(nonMatchB[b, h * NH : (h + 1) * NH], NBLK),
                "mmask": mmask,
                "npad": npad,
            }
        )
    return maps


def kernel(outA, outB, matchA, matchB, nonMatchA, nonMatchB):
    global LAST_EXEC_NS
    from concourse import bass_utils

    nc = _get_nc()
    maps = _in_maps(outA, outB, matchA, matchB, nonMatchA, nonMatchB)

    kwargs = {}
    if os.environ.get("KERNEL_TRACE", "0") == "1":
        kwargs["trace"] = True
    r = bass_utils.run_bass_kernel_spmd(
        nc, maps, core_ids=list(range(NCORES)), **kwargs
    )
    LAST_EXEC_NS = r.exec_time_ns

    partial = np.stack(
        [np.asarray(r.results[c]["out"]).ravel() for c in range(NCORES)]
    )
    match_loss = partial[:, 0].sum(dtype=np.float64) / M_MATCH
    nonmatch_loss = (
        NON_MATCH_WEIGHT * partial[:, 1].sum(dtype=np.float64) / M_NONMATCH
    )
    contrastive = match_loss + nonmatch_loss
    return (
        np.float32(contrastive),
        np.float32(match_loss),
        np.float32(nonmatch_loss),
    )



# revision 2
# speedup vs baseline: 20.1998x; 20.1998x over previous
"""ContrastiveLoss Trainium2 kernel.

Strategy (data-parallel over 8 NeuronCores):
  - 8 cores = 4 batches x 2 halves. Core c handles batch b=c//2, half h=c%2:
    2500 match pairs + 25000 non-match pairs.
  - The A and B descriptor tables are concatenated host-side into one DRAM
    tensor E = [outA[b]; outB[b]] of shape (2N, D), so one indirect DMA can
    gather A-rows and B-rows together (B indices are offset by N).
  - Gather primitive: SWDGE vector-indirect DMA (`indirect_dma_start`) with a
    MULTI-COLUMN offset AP: one instruction with idx tile [128, K] gathers
    128*K rows (row idx[p, j] lands in out[p, j*D:(j+1)*D]). Batching many
    rows per instruction amortizes the ~1us SWDGE fixed overhead that
    dominated the per-block (128-row) gather baseline.
  - Samples are column-blocked: sample s = block j * 128 + partition p.
    Nonmatch runs in NCH chunks of CBLK blocks (A cols then B cols per chunk)
    so DVE/ACT compute on chunk q overlaps the gather of chunk q+1.
  - Per-sample math on DVE/ACT:
      match partial  = sum((mA-mB)^2)              (DVE sub + ACT sq-accum)
      nonmatch partial = sum(relu(0.5-||nA-nB||^2)) (DVE sub, ACT square,
        DVE grouped reduce over D=16, ACT relu with fused accumulate)
  - Padding: tail samples use index 0 on both sides; a host-built {0,1} mask
    zeroes pad match diffs exactly, and a host-built additive bias pushes pad
    nonmatch distances to 1e9 so the hinge is exactly 0.
  - Partition reduction via a ones-vector TensorE matmul into PSUM.
  - Each core outputs [1,2] raw partial sums; the host combines 8x2 scalars
    and applies the 1/5000 and 1/50000 normalizations.

kernel() takes the FULL (unsharded) inputs and returns the full output tuple
(contrastive_loss_sum, match_loss_sum, nonmatch_loss_sum) like the reference.
"""

import os

import numpy as np

# Problem constants (hardcoded per task spec).
B, N, D = 4, 307200, 16
M_MATCH, M_NONMATCH = 5000, 50000
MARGIN = 0.5
NON_MATCH_WEIGHT = 1.0
NCORES = 8

P = 128
MH = M_MATCH // 2  # 2500 match samples per core
NH = M_NONMATCH // 2  # 25000 nonmatch samples per core
MBLK = (MH + P - 1) // P  # 20 match blocks (last one partial: 2500=19*128+68)
NBLK = (NH + P - 1) // P  # 196 nonmatch blocks (25000=195*128+40)
M_REM = MH - (MBLK - 1) * P  # 68 real rows in last match block
N_REM = NH - (NBLK - 1) * P  # 40 real rows in last nonmatch block
CBLK = 49  # nonmatch blocks per gather/compute chunk
NCH = NBLK // CBLK  # 4 chunks
assert NCH * CBLK == NBLK

LAST_EXEC_NS = None

_CACHE = {}


def _build_nc():
    import concourse.bacc as bacc
    import concourse.mybir as mybir
    import concourse.tile as tile
    from concourse import bass

    f32 = mybir.dt.float32
    i32 = mybir.dt.int32
    X = mybir.AxisListType.X
    ADD = mybir.AluOpType.add
    MULT = mybir.AluOpType.mult
    Relu = mybir.ActivationFunctionType.Relu

    nc = bacc.Bacc("TRN2", target_bir_lowering=False, debug=False)
    # concatenated [outA[b]; outB[b]] rows
    eAB = nc.dram_tensor("eAB", (2 * N, D), f32, kind="ExternalInput")
    # nonmatch indices: chunk q occupies cols [q*2C, (q+1)*2C):
    #   first CBLK cols = A-side blocks, next CBLK cols = B-side blocks (+N)
    inI = nc.dram_tensor("inI", (P, 2 * NBLK), i32, kind="ExternalInput")
    # match indices: cols 0..MBLK = A blocks, MBLK..2*MBLK = B blocks (+N)
    imI = nc.dram_tensor("imI", (P, 2 * MBLK), i32, kind="ExternalInput")
    # pad handling: mmask is 1.0 for real match samples else 0.0;
    # npad adds 1e9 to pad nonmatch distances (hinge -> exactly 0)
    mmask = nc.dram_tensor("mmask", (P, MBLK), f32, kind="ExternalInput")
    npad = nc.dram_tensor("npad", (P, CBLK), f32, kind="ExternalInput")
    out = nc.dram_tensor("out", (1, 2), f32, kind="ExternalOutput")

    def gather(dst_ap, src, idx_ap):
        nc.gpsimd.indirect_dma_start(
            out=dst_ap,
            out_offset=None,
            in_=src.ap(),
            in_offset=bass.IndirectOffsetOnAxis(ap=idx_ap, axis=0),
        )

    with tile.TileContext(nc) as tc:
        with (
            tc.tile_pool(name="idx", bufs=1) as idxp,
            tc.tile_pool(name="gath", bufs=2) as gp,
            tc.tile_pool(name="cmp", bufs=2) as cp,
            tc.tile_pool(name="sums", bufs=1) as sp,
            tc.tile_pool(name="psum", bufs=1, space="PSUM") as pp,
        ):
            # index tiles (HWDGE loads; keep Pool free for the gather stream)
            inI_t = idxp.tile([P, 2 * NBLK], i32)
            nc.sync.dma_start(inI_t[:], inI.ap())
            imI_t = idxp.tile([P, 2 * MBLK], i32)
            nc.sync.dma_start(imI_t[:], imI.ap())
            mmask_t = idxp.tile([P, MBLK], f32)
            nc.sync.dma_start(mmask_t[:], mmask.ap())
            npad_t = idxp.tile([P, CBLK], f32)
            nc.sync.dma_start(npad_t[:], npad.ap())

            sums = sp.tile([P, 1 + NCH], f32)
            margin_t = sp.tile([P, 1], f32)
            nc.vector.memset(margin_t[:], MARGIN)

            # --- match: A and B halves in one 40-column gather ---
            mg = gp.tile([P, 2 * MBLK * D], f32, tag="mg")
            gather(mg[:], eAB, imI_t[:])
            md = cp.tile([P, MBLK * D], f32, tag="md")
            nc.vector.tensor_sub(md[:], mg[:, : MBLK * D], mg[:, MBLK * D :])
            # mask the pad samples exactly: mdm = md * mmask (broadcast over D)
            mdm = cp.tile([P, MBLK * D], f32, tag="mdm")
            nc.vector.tensor_tensor(
                out=mdm[:].rearrange("p (s d) -> p s d", d=D),
                in0=md[:].rearrange("p (s d) -> p s d", d=D),
                in1=mmask_t[:].unsqueeze(2).to_broadcast([P, MBLK, D]),
                op=MULT,
            )
            msq = cp.tile([P, MBLK * D], f32, tag="msq")
            nc.scalar.activation(
                msq[:],
                mdm[:],
                mybir.ActivationFunctionType.Square,
                accum_out=sums[:, 0:1],
            )

            # --- nonmatch: gather + compute in chunks of CBLK blocks ---
            for c in range(NCH):
                g = gp.tile([P, 2 * CBLK * D], f32, tag="g")
                gather(g[:], eAB, inI_t[:, c * 2 * CBLK : (c + 1) * 2 * CBLK])

                nd = cp.tile([P, CBLK * D], f32, tag="nd")
                nc.vector.tensor_sub(nd[:], g[:, : CBLK * D], g[:, CBLK * D :])
                nsq = cp.tile([P, CBLK * D], f32, tag="nsq")
                nc.scalar.square(nsq[:], nd[:])
                dist = cp.tile([P, CBLK], f32, tag="dist")
                nc.vector.tensor_reduce(
                    dist[:],
                    nsq[:].rearrange("p (s d) -> p s d", d=D),
                    axis=X,
                    op=ADD,
                )
                if c == NCH - 1:
                    # pad samples: add 1e9 to their distance so the hinge
                    # is exactly 0
                    nc.vector.tensor_add(dist[:], dist[:], npad_t[:])
                hng = cp.tile([P, CBLK], f32, tag="hng")
                nc.scalar.activation(
                    hng[:],
                    dist[:],
                    Relu,
                    bias=margin_t[:],
                    scale=-1.0,
                    accum_out=sums[:, 1 + c : 2 + c],
                )

            # --- cross-partition reduction: ones[128,1].T @ sums[128,1+NCH] ---
            ones = sp.tile([P, 1], f32)
            nc.vector.memset(ones[:], 1.0)
            acc = pp.tile([1, 1 + NCH], f32, space="PSUM")
            nc.tensor.matmul(acc[:], lhsT=ones[:], rhs=sums[:], start=True, stop=True)
            res = sp.tile([1, 2], f32)
            nc.vector.tensor_copy(res[:, 0:1], acc[:, 0:1])
            nc.vector.tensor_reduce(res[:, 1:2], acc[:, 1 : 1 + NCH], axis=X, op=ADD)
            nc.sync.dma_start(out.ap(), res[:])

    nc.compile()
    return nc


def _get_nc():
    if "nc" not in _CACHE:
        _CACHE["nc"] = _build_nc()
    return _CACHE["nc"]


def _blocked(idx_1d, nblocks):
    """[n] -> [128, nblocks] with sample s at [s % 128, s // 128]; pad with 0."""
    out = np.zeros((P, nblocks), np.int32)
    n = idx_1d.shape[0]
    full = n // P
    out[:, :full] = idx_1d[: full * P].reshape(full, P).T
    rem = n - full * P
    if rem:
        out[:rem, full] = idx_1d[full * P :]
    return out


def _in_maps(outA, outB, matchA, matchB, nonMatchA, nonMatchB):
    outA = np.ascontiguousarray(np.asarray(outA, dtype=np.float32))
    outB = np.ascontiguousarray(np.asarray(outB, dtype=np.float32))
    matchA = np.asarray(matchA).astype(np.int32)
    matchB = np.asarray(matchB).astype(np.int32)
    nonMatchA = np.asarray(nonMatchA).astype(np.int32)
    nonMatchB = np.asarray(nonMatchB).astype(np.int32)

    mmask = np.zeros((P, MBLK), np.float32)
    mmask[:, : MBLK - 1] = 1.0
    mmask[:M_REM, MBLK - 1] = 1.0
    npad = np.zeros((P, CBLK), np.float32)
    npad[N_REM:, CBLK - 1] = 1e9

    eAB = [
        np.ascontiguousarray(np.concatenate([outA[b], outB[b]], axis=0))
        for b in range(B)
    ]

    maps = []
    for c in range(NCORES):
        b, h = c // 2, c % 2
        # nonmatch: interleave A/B block-columns chunk by chunk
        ia = _blocked(nonMatchA[b, h * NH : (h + 1) * NH], NBLK)
        ib = _blocked(nonMatchB[b, h * NH : (h + 1) * NH], NBLK) + N
        inI = np.empty((P, 2 * NBLK), np.int32)
        for q in range(NCH):
            inI[:, q * 2 * CBLK : q * 2 * CBLK + CBLK] = (
                ia[:, q * CBLK : (q + 1) * CBLK]
            )
            inI[:, q * 2 * CBLK + CBLK : (q + 1) * 2 * CBLK] = (
                ib[:, q * CBLK : (q + 1) * CBLK]
            )
        ma = _blocked(matchA[b, h * MH : (h + 1) * MH], MBLK)
        mb = _blocked(matchB[b, h * MH : (h + 1) * MH], MBLK) + N
        imI = np.concatenate([ma, mb], axis=1)
        maps.append(
            {
                "eAB": eAB[b],
                "inI": inI,
                "imI": imI,
                "mmask": mmask,
                "npad": npad,
            }
        )
    return maps


def kernel(outA, outB, matchA, matchB, nonMatchA, nonMatchB):
    global LAST_EXEC_NS
    from concourse import bass_utils

    nc = _get_nc()
    maps = _in_maps(outA, outB, matchA, matchB, nonMatchA, nonMatchB)

    kwargs = {}
    if os.environ.get("KERNEL_TRACE", "0") == "1":
        kwargs["trace"] = True
    r = bass_utils.run_bass_kernel_spmd(
        nc, maps, core_ids=list(range(NCORES)), **kwargs
    )
    LAST_EXEC_NS = r.exec_time_ns

    partial = np.stack(
        [np.asarray(r.results[c]["out"]).ravel() for c in range(NCORES)]
    )
    match_loss = partial[:, 0].sum(dtype=np.float64) / M_MATCH
    nonmatch_loss = (
        NON_MATCH_WEIGHT * partial[:, 1].sum(dtype=np.float64) / M_NONMATCH
    )
    contrastive = match_loss + nonmatch_loss
    return (
        np.float32(contrastive),
        np.float32(match_loss),
        np.float32(nonmatch_loss),
    )
